# revision 12
# baseline (speedup 1.0000x reference)
"""DeformableFeatureAggregation on 8 Trainium2 NeuronCores (Bass/Tile).

Sharding: the 4096 anchors are split across the 8 cores (512 each, data-
parallel over queries); the multi-camera feature pyramids (pre-expanded on
the host into 2x2 bilinear-neighborhood rows so one descriptor fetches all
four corners) and the small projection weights are replicated per core.
Nothing is all-gathered: each core writes its own 512 anchors' outputs and
the host concatenates.

Device pipeline per core: PE matmuls for the learnable-scale / softmax-logit
projections, DVE/ACT for keypoint generation + camera projection + softmax,
PE selector-matmuls to wrap gather indices into the 16-partition layout the
SWDGE gather engine expects, batched dma_gather (multiple sample points per
descriptor list) for the bilinear rows, then a bf16 corner-FMA + group-
weight multiply accumulating into 4 round-robin bf16 accumulators (keeps
independent DVE dependency chains in flight), and a final PE projection
with residual add.
"""

import sys
if '/opt/trn_rl_repo' not in sys.path:
    sys.path.insert(0, '/opt/trn_rl_repo')
import numpy as np
import ml_dtypes
from contextlib import ExitStack
import concourse.bass as bass
import concourse.tile as tile
from concourse import bacc, mybir

F32 = mybir.dt.float32
BF16 = mybir.dt.bfloat16
I16 = mybir.dt.int16
I32 = mybir.dt.int32
ALU = mybir.AluOpType
ACTF = mybir.ActivationFunctionType

NA = 512            # anchors per core
NB = 4              # anchor blocks of 128
PTS = 13
CAMS = 6
LVLS = 4
GRP = 8
ED = 256
NWTS = 2496         # 8*312
WL = [176, 88, 44, 22]
HL = [64, 32, 16, 8]
ROWS = [(h + 1) * (w + 1) for h, w in zip(HL, WL)]   # [11505, 2937, 765, 207]

CH4 = [(0, 4), (4, 4), (8, 4), (12, 1)]     # level-0 pt chunks
CH2 = [(0, 2), (2, 2), (4, 2), (6, 2), (8, 2), (10, 2), (12, 1)]  # merged chunks


def build_program(skip_fma=False, skip_gather=False):
    nc = bacc.Bacc("TRN2", target_bir_lowering=False, debug=False, num_devices=8)
    dram = {}
    def din(name, shape, dt):
        dram[name] = nc.dram_tensor(name, shape, dt, kind="ExternalInput").ap()
        return dram[name]

    anchor = din("anchor", [NA, 11], F32)
    iftbf = din("iftbf", [ED, NA], BF16)
    ifeat = din("ifeat", [NA, ED], F32)
    E = [din(f"E{l}", [CAMS * ROWS[l], 1024], BF16) for l in range(LVLS)]
    wlearn = din("wlearn", [ED, 18], BF16)
    blearn = din("blearn", [128, 18], F32)
    wwts = din("wwts", [ED, NWTS], BF16)
    bwts = din("bwts", [128, NWTS], BF16)
    wout = din("wout", [ED, ED], F32)
    bout = din("bout", [128, ED], F32)
    fsc = din("fsc", [128, 21], F32)
    pmat = din("pmat", [128, 96], F32)
    invwh = din("invwh", [128, 12], F32)
    wrap = din("wrap", [128, 8 * 128], F32)
    ident = din("ident", [128, 128], F32)
    camoff = din("camoff", [128, LVLS * CAMS], F32)
    out_d = nc.dram_tensor("out", [NA, ED], F32, kind="ExternalOutput").ap()

    with tile.TileContext(nc) as tc, ExitStack() as ctx:
        cpool = ctx.enter_context(tc.tile_pool(name="const", bufs=1))
        apool = ctx.enter_context(tc.tile_pool(name="aout", bufs=1))
        wpool = ctx.enter_context(tc.tile_pool(name="work", bufs=2))
        tpool = ctx.enter_context(tc.tile_pool(name="tmp78", bufs=3))
        g0pool = ctx.enter_context(tc.tile_pool(name="g0", bufs=12))
        gmpool = ctx.enter_context(tc.tile_pool(name="gm", bufs=4))
        fpool = ctx.enter_context(tc.tile_pool(name="fma", bufs=6))
        ps_big = ctx.enter_context(tc.tile_pool(name="psA", bufs=1, space="PSUM"))
        ps_sm = ctx.enter_context(tc.tile_pool(name="psS", bufs=1, space="PSUM"))
        ps_w = ctx.enter_context(tc.tile_pool(name="psW", bufs=2, space="PSUM"))

        def ctile(ap_src, shape, dt, tag):
            t = cpool.tile(shape, dt, tag=tag)
            nc.sync.dma_start(t[:], ap_src)
            return t

        # ---- persistent constants in SBUF ----
        iftbf_t = [ctile(iftbf[k * 128:(k + 1) * 128, :], [128, NA], BF16, f"iftbf_{k}") for k in range(2)]
        wl_t = [ctile(wlearn[k * 128:(k + 1) * 128, :], [128, 18], BF16, f"wl_{k}") for k in range(2)]
        bl_t = ctile(blearn[:, :], [128, 18], F32, "bl")
        ww_t = [ctile(wwts[k * 128:(k + 1) * 128, :], [128, NWTS], BF16, f"ww_{k}") for k in range(2)]
        bw_t = ctile(bwts[:, :], [128, NWTS], BF16, "bwts")
        wo_t = [ctile(wout[k * 128:(k + 1) * 128, :], [128, ED], F32, f"wo_{k}") for k in range(2)]
        bo_t = ctile(bout[:, :], [128, ED], F32, "bo")
        fs_t = ctile(fsc[:, :], [128, 21], F32, "fs")
        pm_t = ctile(pmat[:, :], [128, 96], F32, "pm")
        iw_t = ctile(invwh[:, :], [128, 12], F32, "iw")
        wrap_t = ctile(wrap[:, :], [128, 8, 128], F32, "wrap")
        id_t = ctile(ident[:, :], [128, 128], F32, "id")
        co_t = ctile(camoff[:, :], [128, LVLS, CAMS], F32, "camoff")

        # ---- stage A persistent outputs (all blocks) ----
        wn_b = [apool.tile([128, GRP, 312], BF16, tag=f"wn{b}", name=f"wn{b}") for b in range(NB)]
        bwp_b = [apool.tile([128, LVLS, CAMS, 4, PTS], F32, tag=f"bwp{b}", name=f"bwp{b}") for b in range(NB)]
        # gather-ready index layouts: level-0 (c, pt, s) / merged (l-1, pt, c, s)
        i0_b = [apool.tile([128, CAMS, PTS, 8], I16, tag=f"i0_{b}", name=f"i0_{b}") for b in range(NB)]
        im_b = [apool.tile([128, 3, PTS, CAMS, 8], I16, tag=f"im_{b}", name=f"im_{b}") for b in range(NB)]
        acc_b = [apool.tile([128, 4, ED], BF16, tag=f"acc{b}", name=f"acc{b}") for b in range(NB)]

        V = nc.vector
        SC = nc.scalar
        g0dum = gmdum = None
        if skip_gather:
            g0dum = cpool.tile([128, 4, 1024], BF16, tag="g0dum", name="g0dum")
            nc.vector.memset(g0dum[:], 0.0)
            gmdum = cpool.tile([128, 12, 1024], BF16, tag="gmdum", name="gmdum")
            nc.vector.memset(gmdum[:], 0.0)

        # =============== STAGE A ===============
        for b in range(NB):
            a0 = b * 128
            anc = wpool.tile([128, 11], F32, tag="anc", name="anc")
            nc.sync.dma_start(anc[:], anchor[a0:a0 + 128, :])

            # learnable scale: L = IF @ W_learn + b_learn ; sigmoid - 0.5
            Lp = ps_sm.tile([128, 18], F32, tag="pss", name="Lp")
            for k in range(2):
                nc.tensor.matmul(Lp[:], iftbf_t[k][:, a0:a0 + 128], wl_t[k][:],
                                 start=(k == 0), stop=(k == 1))
            Lb = wpool.tile([128, 18], F32, tag="Lb", name="Lb")
            V.tensor_add(Lb[:], Lp[:], bl_t[:])
            SC.activation(Lb[:], Lb[:], ACTF.Sigmoid)
            scale3 = wpool.tile([128, 39], F32, tag="scale3", name="scale3")
            V.tensor_copy(scale3[:, 0:21], fs_t[:])
            V.tensor_scalar_sub(scale3[:, 21:39], Lb[:], 0.5)

            # kp0 = scale3 * gs_scales (broadcast xyz)
            kp0 = wpool.tile([128, PTS, 3], F32, tag="kp0", name="kp0")
            gs_b = anc[:, 3:6].unsqueeze(1).broadcast_to([128, PTS, 3])
            V.tensor_tensor(kp0[:], scale3[:].rearrange("p (t x) -> p t x", x=3), gs_b, ALU.mult)

            # rotation from quaternion
            qq = wpool.tile([128, 16], F32, tag="qq", name="qq")
            q_i = anc[:, 6:10].unsqueeze(2).broadcast_to([128, 4, 4])
            q_j = anc[:, 6:10].unsqueeze(1).broadcast_to([128, 4, 4])
            V.tensor_tensor(qq[:].rearrange("p (i j) -> p i j", i=4), q_i, q_j, ALU.mult)
            sq4 = wpool.tile([128, 4], F32, tag="sq4", name="sq4")
            V.tensor_tensor(sq4[:], anc[:, 6:10], anc[:, 6:10], ALU.mult)
            n2 = wpool.tile([128, 1], F32, tag="n2", name="n2")
            V.tensor_reduce(n2[:], sq4[:], mybir.AxisListType.X, ALU.add)
            inv2 = wpool.tile([128, 1], F32, tag="inv2", name="inv2")
            V.reciprocal(inv2[:], n2[:])
            V.tensor_scalar_mul(qq[:], qq[:], inv2[:, 0:1])
            # R entries
            R = wpool.tile([128, 9], F32, tag="R", name="R")
            tmp1 = wpool.tile([128, 1], F32, tag="tmp1", name="tmp1")
            for i, (ca, cb) in enumerate([(10, 15), (5, 15), (5, 10)]):
                V.tensor_add(tmp1[:], qq[:, ca:ca + 1], qq[:, cb:cb + 1])
                V.tensor_scalar(R[:, 4 * i:4 * i + 1], tmp1[:], -2.0, 1.0, ALU.mult, ALU.add)
            for (ca, cb, op, d) in [(6, 3, ALU.subtract, 1), (7, 2, ALU.add, 2),
                                    (6, 3, ALU.add, 3), (11, 1, ALU.subtract, 5),
                                    (7, 2, ALU.subtract, 6), (11, 1, ALU.add, 7)]:
                V.tensor_tensor(tmp1[:], qq[:, ca:ca + 1], qq[:, cb:cb + 1], op)
                V.tensor_scalar_mul(R[:, d:d + 1], tmp1[:], 2.0)

            # kp rotated + translated
            kpr = wpool.tile([128, 3, PTS], F32, tag="kpr", name="kpr")
            t13a = wpool.tile([128, PTS], F32, tag="t13a", name="t13a")
            for i in range(3):
                V.tensor_scalar_mul(t13a[:], kp0[:, :, 0], R[:, i:i + 1])
                V.scalar_tensor_tensor(t13a[:], kp0[:, :, 1], R[:, 3 + i:4 + i], t13a[:], ALU.mult, ALU.add)
                V.scalar_tensor_tensor(t13a[:], kp0[:, :, 2], R[:, 6 + i:7 + i], t13a[:], ALU.mult, ALU.add)
                V.tensor_scalar_add(kpr[:, i, :], t13a[:], anc[:, i:i + 1])

            # projection per cam -> xhat/yhat in [0, 0.9999]
            xh = wpool.tile([128, CAMS, PTS], F32, tag="xh", name="xh")
            yh = wpool.tile([128, CAMS, PTS], F32, tag="yh", name="yh")
            for c in range(CAMS):
                pc = c * 16
                rows = []
                for i in range(3):
                    ti = tpool.tile([128, PTS], F32, tag=f"proj{i}", name=f"proj{i}")
                    V.tensor_scalar_mul(ti[:], kpr[:, 0, :], pm_t[:, pc + 4 * i:pc + 4 * i + 1])
                    V.scalar_tensor_tensor(ti[:], kpr[:, 1, :], pm_t[:, pc + 4 * i + 1:pc + 4 * i + 2], ti[:], ALU.mult, ALU.add)
                    V.scalar_tensor_tensor(ti[:], kpr[:, 2, :], pm_t[:, pc + 4 * i + 2:pc + 4 * i + 3], ti[:], ALU.mult, ALU.add)
                    V.tensor_scalar_add(ti[:], ti[:], pm_t[:, pc + 4 * i + 3:pc + 4 * i + 4])
                    rows.append(ti)
                X, Y, Z = rows
                V.tensor_single_scalar(Z[:], Z[:], 1e-5, ALU.max)
                Zi = tpool.tile([128, PTS], F32, tag="Zi", name="Zi")
                V.reciprocal(Zi[:], Z[:])
                for (src, dst, col) in [(X, xh, 2 * c), (Y, yh, 2 * c + 1)]:
                    V.tensor_tensor(t13a[:], src[:], Zi[:], ALU.mult)
                    V.tensor_scalar_mul(t13a[:], t13a[:], iw_t[:, col:col + 1])
                    V.tensor_scalar(dst[:, c, :], t13a[:], 0.0, 0.9999, ALU.max, ALU.min)

            # per level (batched over cams): bilinear weights + cell index
            NC6 = CAMS * PTS  # 78
            idxf = wpool.tile([128, LVLS, CAMS, PTS], F32, tag="idxf", name="idxf")
            px1 = tpool.tile([128, CAMS, PTS], F32, tag="px1", name="px1")
            py1 = tpool.tile([128, CAMS, PTS], F32, tag="py1", name="py1")
            wxt = tpool.tile([128, CAMS, PTS], F32, tag="wxt", name="wxt")
            wyt = tpool.tile([128, CAMS, PTS], F32, tag="wyt", name="wyt")
            oxt = tpool.tile([128, CAMS, PTS], F32, tag="oxt", name="oxt")
            oyt = tpool.tile([128, CAMS, PTS], F32, tag="oyt", name="oyt")
            xbt = tpool.tile([128, CAMS, PTS], F32, tag="xbt", name="xbt")
            ybt = tpool.tile([128, CAMS, PTS], F32, tag="ybt", name="ybt")
            t78 = tpool.tile([128, CAMS, PTS], F32, tag="t78", name="t78")
            ifix = tpool.tile([128, CAMS, PTS], I32, tag="ifix", name="ifix")
            for l in range(LVLS):
                V.tensor_scalar(px1[:], xh[:], float(WL[l]), 0.5, ALU.mult, ALU.add)
                V.tensor_scalar(py1[:], yh[:], float(HL[l]), 0.5, ALU.mult, ALU.add)
                for (p1, bt, wt) in ((px1, xbt, wxt), (py1, ybt, wyt)):
                    V.tensor_copy(ifix[:], p1[:])          # f32 -> int32
                    V.tensor_copy(bt[:], ifix[:])          # back to f32
                    V.tensor_sub(wt[:], p1[:], bt[:])      # err (may be <0 if cast rounded up)
                    V.tensor_single_scalar(t78[:], wt[:], 0.0, ALU.is_lt)
                    V.tensor_sub(bt[:], bt[:], t78[:])     # floor
                    V.tensor_sub(wt[:], p1[:], bt[:])      # frac in [0,1)
                V.tensor_scalar(oxt[:], wxt[:], -1.0, 1.0, ALU.mult, ALU.add)
                V.tensor_scalar(oyt[:], wyt[:], -1.0, 1.0, ALU.mult, ALU.add)
                # corners: 0:(y0,x0) 1:(y0,x1) 2:(y1,x0) 3:(y1,x1)
                V.tensor_tensor(bwp_b[b][:, l, :, 0, :], oyt[:], oxt[:], ALU.mult)
                V.tensor_tensor(bwp_b[b][:, l, :, 1, :], oyt[:], wxt[:], ALU.mult)
                V.tensor_tensor(bwp_b[b][:, l, :, 2, :], wyt[:], oxt[:], ALU.mult)
                V.tensor_tensor(bwp_b[b][:, l, :, 3, :], wyt[:], wxt[:], ALU.mult)
                # cell index: yb*(W+1)+xb + cam offset (merged levels)
                V.scalar_tensor_tensor(t78[:], ybt[:], float(WL[l] + 1), xbt[:], ALU.mult, ALU.add)
                if l == 0:
                    V.tensor_copy(idxf[:, l, :, :], t78[:])
                else:
                    cob = co_t[:, l, :].unsqueeze(2).broadcast_to([128, CAMS, PTS])
                    V.tensor_tensor(idxf[:, l, :, :], t78[:], cob, ALU.add)

            # weights: logits = IF @ W_wts + b_wts ; softmax over 312 per group
            Wp = ps_big.tile([128, NWTS], F32, tag="Wp", name="Wp")
            for k in range(2):
                for n0 in range(0, NWTS, 512):
                    n1 = min(n0 + 512, NWTS)
                    nc.tensor.matmul(Wp[:, n0:n1], iftbf_t[k][:, a0:a0 + 128], ww_t[k][:, n0:n1],
                                     start=(k == 0), stop=(k == 1))
            wnb = wn_b[b]
            wn_flat = wnb[:].rearrange("p g j -> p (g j)")
            V.tensor_add(wn_flat, Wp[:], bw_t[:])
            SC.activation(wn_flat, wn_flat, ACTF.Exp)
            ssum = wpool.tile([128, GRP], F32, tag="ssum", name="ssum")
            V.tensor_reduce(ssum[:], wnb[:], mybir.AxisListType.X, ALU.add)
            sinv = wpool.tile([128, GRP], F32, tag="sinv", name="sinv")
            V.reciprocal(sinv[:], ssum[:])
            V.tensor_tensor(wnb[:], wnb[:], sinv[:].unsqueeze(2).broadcast_to([128, GRP, 312]), ALU.mult)

            V.memset(acc_b[b][:], 0.0)

            # index wrap: per pt, 8 PE selector matmuls -> one PSUM -> two copies
            for pt in range(PTS):
                pw8 = ps_w.tile([128, 8, 24], F32, tag="pw8", name="pw8")
                mv = idxf[:, :, :, pt].rearrange("p l c -> p (l c)")
                for s in range(8):
                    nc.tensor.matmul(pw8[:, s, :], wrap_t[:, s, :], mv, start=True, stop=True)
                V.tensor_copy(i0_b[b][:, :, pt, :],
                              pw8[:, :, 0:6].rearrange("p s c -> p c s"))
                V.tensor_copy(im_b[b][:, :, pt, :, :],
                              pw8[:, :, 6:24].rearrange("p s (l c) -> p l c s", c=6))

        # =============== STAGE B: gather + FMA ===============
        for b in range(NB):
            it = 0
            for pt in range(PTS):
                g0t = {}
                for c in range(CAMS):
                    if skip_gather:
                        g0t[c] = g0dum
                        continue
                    g0 = g0pool.tile([128, 1, 1024], BF16, tag="g0", name="g0")
                    nc.gpsimd.dma_gather(
                        out_ap=g0[:], in_ap=E[0][c * ROWS[0]:(c + 1) * ROWS[0], :],
                        idxs_ap=i0_b[b][:, c, pt, :],
                        num_idxs=128, num_idxs_reg=128, elem_size=1024)
                    g0t[c] = g0
                gmt = {}
                for l in range(1, LVLS):
                    if skip_gather:
                        gmt[l] = gmdum
                        continue
                    gm = gmpool.tile([128, 6, 1024], BF16, tag="gm", name="gm")
                    nc.gpsimd.dma_gather(
                        out_ap=gm[:], in_ap=E[l][:, :],
                        idxs_ap=im_b[b][:, l - 1, pt, :, :],
                        num_idxs=768, num_idxs_reg=768, elem_size=1024)
                    gmt[l] = gm
                if skip_fma:
                    continue
                for c in range(CAMS):
                    for l in range(LVLS):
                        if l == 0:
                            gap = g0t[c][:, 0, :] if not skip_gather else g0dum[:, 0, :]
                        else:
                            gap = gmt[l][:, c, :] if not skip_gather else gmdum[:, c, :]
                        T = fpool.tile([128, ED], BF16, tag="T", name="T")
                        bwap = bwp_b[b][:, l, c, :, pt:pt + 1]
                        V.tensor_scalar_mul(T[:], gap[:, 0:256], bwap[:, 0, :])
                        for k in range(1, 4):
                            V.scalar_tensor_tensor(T[:], gap[:, 256 * k:256 * (k + 1)], bwap[:, k, :],
                                                   T[:], ALU.mult, ALU.add)
                        P = fpool.tile([128, ED], BF16, tag="P", name="P")
                        j0 = c * 52 + l * 13 + pt
                        wv = wn_b[b][:, :, j0].unsqueeze(1).broadcast_to([128, 32, GRP])
                        V.tensor_tensor(P[:].rearrange("p (d g) -> p d g", d=32),
                                        T[:].rearrange("p (d g) -> p d g", d=32), wv, ALU.mult)
                        a = acc_b[b][:, it % 4, :]
                        V.tensor_add(a, a, P[:])
                        it += 1

        # =============== STAGE C: combine + output projection ===============
        for b in range(NB):
            a0 = b * 128
            S01 = wpool.tile([128, ED], F32, tag="S01", name="S01")
            S23 = wpool.tile([128, ED], F32, tag="S23", name="S23")
            V.tensor_add(S01[:], acc_b[b][:, 0, :], acc_b[b][:, 1, :])
            V.tensor_add(S23[:], acc_b[b][:, 2, :], acc_b[b][:, 3, :])
            S = wpool.tile([128, ED], F32, tag="S", name="S")
            V.tensor_add(S[:], S01[:], S23[:])
            FT = wpool.tile([128, 2, 128], F32, tag="FT", name="FT")
            for k in range(2):
                pt_ = ps_sm.tile([128, 128], F32, tag="pss", name="ptT")
                nc.tensor.transpose(pt_[:], S[:, 128 * k:128 * (k + 1)], id_t[:])
                V.tensor_copy(FT[:, k, :], pt_[:])
            Op = ps_sm.tile([128, ED], F32, tag="pss", name="Op")
            for k in range(2):
                nc.tensor.matmul(Op[:], FT[:, k, :], wo_t[k][:], start=(k == 0), stop=(k == 1))
            ifl = wpool.tile([128, ED], F32, tag="ifl", name="ifl")
            nc.sync.dma_start(ifl[:], ifeat[a0:a0 + 128, :])
            Ob = wpool.tile([128, ED], F32, tag="Ob", name="Ob")
            V.tensor_add(Ob[:], Op[:], bo_t[:])
            V.tensor_add(Ob[:], Ob[:], ifl[:])
            nc.sync.dma_start(out_d[a0:a0 + 128, :], Ob[:])

    nc.compile()
    return nc


# channel permutation: new position d*8+g holds original channel g*32+d
CH_PERM = np.array([g * 32 + d for d in range(32) for g in range(8)], dtype=np.int64)


def host_prep(inputs):
    """Build per-core in_maps from full inputs dict."""
    IF = np.asarray(inputs["instance_feature"][0])      # [4096, 256]
    AN = np.asarray(inputs["anchor"][0])                # [4096, 11]
    PM = np.asarray(inputs["projection_mat"][0])        # [6, 4, 4]
    IW = np.asarray(inputs["image_wh"][0])              # [6, 2]
    Wl = np.asarray(inputs["W_learn"]); bl = np.asarray(inputs["b_learn"])
    Ww = np.asarray(inputs["W_wts"]); bw = np.asarray(inputs["b_wts"])
    Wo = np.asarray(inputs["W_out"]); bo = np.asarray(inputs["b_out"])
    feats = [np.asarray(inputs[f"feat{l}"][0]) for l in range(4)]  # [6, 256, H, W]

    ones = np.ones((128, 1), np.float32)
    # expanded neighborhood maps, channel-permuted, bf16
    Emaps = []
    for l, fm in enumerate(feats):
        fmp = fm[:, CH_PERM]                             # [6, 256, H, W] perm'd
        H, W = HL[l], WL[l]
        Mp = np.zeros((CAMS, H + 2, W + 2, 256), np.float32)
        Mp[:, 1:H + 1, 1:W + 1, :] = np.transpose(fmp, (0, 2, 3, 1))
        Eh = np.concatenate([Mp[:, :-1, :-1], Mp[:, :-1, 1:], Mp[:, 1:, :-1], Mp[:, 1:, 1:]], axis=-1)
        Emaps.append(np.ascontiguousarray(Eh.reshape(CAMS * ROWS[l], 1024)).astype(ml_dtypes.bfloat16))

    # W_wts column permutation to g-major: new col g*312 + j <- old col j*8 + g
    j = np.arange(312)
    colperm = np.empty(NWTS, np.int64)
    for g in range(8):
        colperm[g * 312 + j] = j * 8 + g
    Ww_p = np.ascontiguousarray(Ww[:, colperm]).astype(ml_dtypes.bfloat16)
    bw_p = bw[colperm].astype(np.float32)

    Wo_p = np.ascontiguousarray(Wo[CH_PERM, :]).astype(np.float32)

    FIX = np.array([[0, 0, 0], [0.45, 0, 0], [-0.45, 0, 0], [0, 0.45, 0],
                    [0, -0.45, 0], [0, 0, 0.45], [0, 0, -0.45]], np.float32)

    wrap_h = np.zeros((128, 8, 128), np.float32)
    for s in range(8):
        for m in range(128):
            wrap_h[s * 16 + (m % 16), s, m] = 1.0

    camoff_h = np.zeros((LVLS, CAMS), np.float32)
    for l in range(1, LVLS):
        camoff_h[l] = np.arange(CAMS) * ROWS[l]

    shared = {
        "wlearn": Wl.astype(ml_dtypes.bfloat16), "blearn": ones * 0 + bl[None, :].astype(np.float32),
        "wwts": Ww_p, "bwts": (ones * 0 + bw_p[None, :]).astype(ml_dtypes.bfloat16),
        "wout": Wo_p, "bout": ones * 0 + bo[None, :].astype(np.float32),
        "fsc": np.tile(FIX.reshape(1, 21), (128, 1)).astype(np.float32),
        "pmat": np.tile(PM.reshape(1, 96), (128, 1)).astype(np.float32),
        "invwh": np.tile((1.0 / IW).reshape(1, 12), (128, 1)).astype(np.float32),
        "wrap": wrap_h.reshape(128, 8 * 128),
        "ident": np.eye(128, dtype=np.float32),
        "camoff": np.tile(camoff_h.reshape(1, LVLS * CAMS), (128, 1)).astype(np.float32),
    }
    for l in range(4):
        shared[f"E{l}"] = Emaps[l]

    in_maps = []
    for core in range(8):
        s = slice(core * NA, (core + 1) * NA)
        m = dict(shared)
        m["anchor"] = np.ascontiguousarray(AN[s]).astype(np.float32)
        ifc = np.ascontiguousarray(IF[s]).astype(np.float32)
        m["ifeat"] = ifc
        ift = np.ascontiguousarray(ifc.T)
        m["iftbf"] = ift.astype(ml_dtypes.bfloat16)
        in_maps.append(m)
    return in_maps


def assemble(results):
    return np.concatenate([results[c]["out"] for c in range(8)], axis=0)[None]


def kernel(**inputs):
    from concourse.bass_utils import run_bass_kernel_spmd
    nc = build_program()
    in_maps = host_prep(inputs)
    res = run_bass_kernel_spmd(nc, in_maps, list(range(8))).results
    return assemble(res).astype(np.float32)
